# revision 8
# baseline (speedup 1.0000x reference)
"""Single-head attention layer on 8 NeuronCores, data-parallel over batch.

Per core (one batch): x [T, D] with T=2048, D=1024.
    q = x@Wq.T, k = x@Wk.T, v = x@Wv.T
    score = q@k.T / sqrt(T); attn = softmax(score); out = (attn@v)@Wo.T

Everything on-chip is kept feature-major (transposed), so no transposes are
ever needed on the device (the host pre-transposes x and the weights, and
re-transposes the output):
    qT[h,t] = wqT.T @ xT          kT[h,s] = wkT.T @ xT
    v[s,h]  = xT.T @ wvT          scoreT[s,t] = kT_slice.T @ qT
    expT    = exp(scoreT/sqrt(T))            (no max subtraction: |score/sqrt(T)|<~5)
    denom   = ones.T @ expT                  (softmax denominator, broadcast on all
                                              partitions via a rank-128 ones matmul)
    oT[h,t] = v_slice.T @ expT;  oT *= 1/denom
    outT[o,t] = woT_slice.T @ oT

All matmul operands are bf16 (inputs are DMA-cast fp32->bf16 on load), so
every matmul runs at 1 cycle/row with fast weight load; accumulation is fp32
in PSUM and the softmax normalization arithmetic is fp32.
"""

import numpy as np

P = 128


def _build_attention(tc, aps, D, T, TB, CH):
    """Emit the per-core attention kernel into TileContext `tc`.

    aps: dict with DRAM APs xT[D,T], wqT/wkT/wvT[D,D] ([x,h]), woT[D,D] ([h,o]),
         outT[D,T] ([o,t]).
    TB: t-block size for the attention phase. CH: x-streaming chunk size.
    """
    from contextlib import ExitStack

    import concourse.mybir as mybir
    from concourse.bass import ts

    nc = tc.nc
    fp32 = mybir.dt.float32
    bf16 = mybir.dt.bfloat16
    Exp = mybir.ActivationFunctionType.Exp

    XO = D // P          # x (contraction) tiles
    HO = D // P          # h tiles
    SO = T // P          # s tiles
    NTB = T // TB        # t blocks
    NCH = T // CH        # x-stream chunks over t/s
    VH = min(512, D)     # v-proj h chunk
    NVH = D // VH
    SCALE = float(1.0 / np.sqrt(np.float32(T)))

    xT, wqT, wkT, wvT, woT, outT = (
        aps["xT"], aps["wqT"], aps["wkT"], aps["wvT"], aps["woT"], aps["outT"],
    )

    with ExitStack() as top:
        persist = top.enter_context(tc.tile_pool(name="persist", bufs=1))

        kT = persist.tile([P, HO, T], bf16, name="kT", tag="kT")
        qT = persist.tile([P, HO, T], bf16, name="qT", tag="qT")
        vsb = persist.tile([P, SO, D], bf16, name="vsb", tag="vsb")
        ones = persist.tile([P, P], bf16, name="ones", tag="ones")
        nc.vector.memset(ones[:], 1.0)

        # ---------------- phase 1: q/k/v projections (x streamed once) ------
        # Inputs arrive bf16 from the host; weight loads are interleaved with
        # the first chunk's matmuls so the PE starts as early as possible.
        with ExitStack() as ph1:
            xs_pool = ph1.enter_context(tc.tile_pool(name="xs", bufs=2))
            w_pool = ph1.enter_context(tc.tile_pool(name="w", bufs=1))
            ps1 = ph1.enter_context(tc.tile_pool(name="ps1", bufs=4, space="PSUM"))

            wq = [w_pool.tile([P, D], bf16, name=f"wq{x}", tag=f"wq{x}") for x in range(XO)]
            wk = [w_pool.tile([P, D], bf16, name=f"wk{x}", tag=f"wk{x}") for x in range(XO)]
            wv = [w_pool.tile([P, D], bf16, name=f"wv{x}", tag=f"wv{x}") for x in range(XO)]
            H2 = D // 2
            for x in range(XO):
                nc.sync.dma_start(wq[x][:, :H2], wqT[ts(x, P), :H2])
                nc.sync.dma_start(wq[x][:, H2:], wqT[ts(x, P), H2:])

            def proj_qk(dst, w, xts, i):
                for h in range(HO):
                    ps = ps1.tile([P, CH], fp32, name="ps_qk", tag="ps")
                    for x in range(XO):
                        nc.tensor.matmul(
                            ps[:], w[x][:, ts(h, P)], xts[:, x, :],
                            start=(x == 0), stop=(x == XO - 1),
                        )
                    nc.scalar.copy(dst[:, h, ts(i, CH)], ps[:])

            def proj_v(xts, i):
                for sl in range(CH // P):
                    s = i * (CH // P) + sl
                    for hc in range(NVH):
                        vps = ps1.tile([P, VH], fp32, name="vps", tag="ps")
                        for x in range(XO):
                            nc.tensor.matmul(
                                vps[:], xts[:, x, ts(sl, P)], wv[x][:, ts(hc, VH)],
                                start=(x == 0), stop=(x == XO - 1),
                            )
                        nc.vector.tensor_copy(vsb[:, s, ts(hc, VH)], vps[:])

            for i in range(NCH):
                xts = xs_pool.tile([P, XO, CH], bf16, name=f"xs{i}", tag="xs")
                C2 = CH // 2
                for x in range(XO):
                    nc.sync.dma_start(xts[:, x, :C2], xT[ts(x, P), i * CH:i * CH + C2])
                    nc.sync.dma_start(xts[:, x, C2:], xT[ts(x, P), i * CH + C2:(i + 1) * CH])
                proj_qk(qT, wq, xts, i)
                if i == 0:
                    for x in range(XO):
                        nc.sync.dma_start(wk[x][:, :H2], wkT[ts(x, P), :H2])
                        nc.sync.dma_start(wk[x][:, H2:], wkT[ts(x, P), H2:])
                proj_qk(kT, wk, xts, i)
                if i == 0:
                    for x in range(XO):
                        nc.sync.dma_start(wv[x][:, :H2], wvT[ts(x, P), :H2])
                        nc.sync.dma_start(wv[x][:, H2:], wvT[ts(x, P), H2:])
                proj_v(xts, i)

        # ---------------- phase 2: attention + output projection ------------
        with ExitStack() as ph2:
            wo_pool = ph2.enter_context(tc.tile_pool(name="wo", bufs=1))
            exp_pool = ph2.enter_context(tc.tile_pool(name="expp", bufs=SO + 2))
            ot_pool = ph2.enter_context(tc.tile_pool(name="ot", bufs=HO + 2))
            out_pool = ph2.enter_context(tc.tile_pool(name="outp", bufs=4))
            rc_pool = ph2.enter_context(tc.tile_pool(name="rc", bufs=2))
            ps_s = ph2.enter_context(tc.tile_pool(name="pss", bufs=2, space="PSUM"))
            ps_d = ph2.enter_context(tc.tile_pool(name="psd", bufs=1, space="PSUM"))
            ps_o = ph2.enter_context(tc.tile_pool(name="pso", bufs=2, space="PSUM"))
            ps_w = ph2.enter_context(tc.tile_pool(name="psw", bufs=2, space="PSUM"))

            wo = [wo_pool.tile([P, D], bf16, name=f"wo{h}", tag=f"wo{h}") for h in range(HO)]
            for h in range(HO):
                nc.sync.dma_start(wo[h][:], woT[ts(h, P), :])

            for tb in range(NTB):
                # scores + exp + denominator, s-tile at a time
                den_ps = ps_d.tile([P, TB], fp32, name="denps", tag="denps")
                exps = []
                for s in range(SO):
                    sps = ps_s.tile([P, TB], fp32, name="sps", tag="sps")
                    for h in range(HO):
                        nc.tensor.matmul(
                            sps[:], kT[:, h, ts(s, P)], qT[:, h, ts(tb, TB)],
                            start=(h == 0), stop=(h == HO - 1),
                        )
                    et = exp_pool.tile([P, TB], bf16, name=f"exp{s}", tag="exp")
                    nc.scalar.activation(et[:], sps[:], Exp, scale=SCALE)
                    exps.append(et)
                    nc.tensor.matmul(
                        den_ps[:], ones[:], et[:],
                        start=(s == 0), stop=(s == SO - 1),
                    )

                recip = rc_pool.tile([P, TB], fp32, name="recip", tag="recip")
                nc.vector.reciprocal(recip[:], den_ps[:])

                # oT[h,:] = sum_s v[s,h-slice].T @ expT[s], then normalize
                ots = []
                for h in range(HO):
                    ops = ps_o.tile([P, TB], fp32, name="ops", tag="ops")
                    for s in range(SO):
                        nc.tensor.matmul(
                            ops[:], vsb[:, s, ts(h, P)], exps[s][:],
                            start=(s == 0), stop=(s == SO - 1),
                        )
                    ot = ot_pool.tile([P, TB], bf16, name=f"ot{h}", tag="ot")
                    nc.vector.tensor_mul(ot[:], ops[:], recip[:])
                    ots.append(ot)

                # output projection: outT[o,:] = sum_h woT[h,o-slice].T @ oT[h]
                for o in range(HO):
                    wps = ps_w.tile([P, TB], fp32, name="wps", tag="wps")
                    for h in range(HO):
                        nc.tensor.matmul(
                            wps[:], wo[h][:, ts(o, P)], ots[h][:],
                            start=(h == 0), stop=(h == HO - 1),
                        )
                    osb = out_pool.tile([P, TB], fp32, name="osb", tag="osb")
                    nc.scalar.copy(osb[:], wps[:])
                    nc.sync.dma_start(outT[ts(o, P), ts(tb, TB)], osb[:])


def build_bass(D=1024, T=2048, TB=512, CH=512):
    import concourse.mybir as mybir
    import concourse.tile as tile
    from concourse import bacc

    fp32 = mybir.dt.float32
    bf16 = mybir.dt.bfloat16
    nc = bacc.Bacc("TRN2", debug=False)
    aps = {
        "xT": nc.dram_tensor("xT", [D, T], bf16, kind="ExternalInput")[:],
        "wqT": nc.dram_tensor("wqT", [D, D], bf16, kind="ExternalInput")[:],
        "wkT": nc.dram_tensor("wkT", [D, D], bf16, kind="ExternalInput")[:],
        "wvT": nc.dram_tensor("wvT", [D, D], bf16, kind="ExternalInput")[:],
        "woT": nc.dram_tensor("woT", [D, D], bf16, kind="ExternalInput")[:],
        "outT": nc.dram_tensor("outT", [D, T], fp32, kind="ExternalOutput")[:],
    }
    with tile.TileContext(nc) as tc:
        _build_attention(tc, aps, D=D, T=T, TB=TB, CH=CH)
    nc.compile()
    return nc


def kernel(x, W_q, W_k, W_v, W_o):
    from concourse import bass_utils

    import ml_dtypes

    bf16 = ml_dtypes.bfloat16
    x = np.asarray(x, dtype=np.float32)
    B = x.shape[0]
    wqT = np.ascontiguousarray(np.asarray(W_q, np.float32).T.astype(bf16))
    wkT = np.ascontiguousarray(np.asarray(W_k, np.float32).T.astype(bf16))
    wvT = np.ascontiguousarray(np.asarray(W_v, np.float32).T.astype(bf16))
    woT = np.ascontiguousarray(np.asarray(W_o, np.float32).T.astype(bf16))

    in_maps = [
        {
            "xT": np.ascontiguousarray(x[b].T.astype(bf16)),
            "wqT": wqT,
            "wkT": wkT,
            "wvT": wvT,
            "woT": woT,
        }
        for b in range(B)
    ]

    nc = build_bass()
    res = bass_utils.run_bass_kernel_spmd(nc, in_maps, core_ids=list(range(B)))
    out = np.stack([res.results[b]["outT"].T for b in range(B)])
    return np.ascontiguousarray(out.astype(np.float32))


# revision 10
# speedup vs baseline: 1.0338x; 1.0338x over previous
"""Single-head attention layer on 8 NeuronCores, data-parallel over batch.

Per core (one batch): x [T, D] with T=2048, D=1024.
    q = x@Wq.T, k = x@Wk.T, v = x@Wv.T
    score = q@k.T / sqrt(T); attn = softmax(score); out = (attn@v)@Wo.T

Everything on-chip is kept feature-major (transposed), so no transposes are
ever needed on the device (the host pre-transposes x and the weights, and
re-transposes the output):
    qT[h,t] = wqT.T @ xT          kT[h,s] = wkT.T @ xT
    v[s,h]  = xT.T @ wvT          scoreT[s,t] = kT_slice.T @ qT
    expT    = exp(scoreT/sqrt(T))            (no max subtraction: |score/sqrt(T)|<~5)
    denom   = partition_all_reduce(sum_s expT)   (softmax denominator on DVE+GpSimd,
                                                  result broadcast on all partitions)
    oT[h,t] = v_slice.T @ expT;  oT *= 1/denom
    outT[o,t] = woT_slice.T @ oT

All matmul operands are bf16 (inputs are DMA-cast fp32->bf16 on load), so
every matmul runs at 1 cycle/row with fast weight load; accumulation is fp32
in PSUM and the softmax normalization arithmetic is fp32.
"""

import numpy as np

P = 128


def _build_attention(tc, aps, D, T, TB, CH):
    """Emit the per-core attention kernel into TileContext `tc`.

    aps: dict with DRAM APs xT[D,T], wqT/wkT/wvT[D,D] ([x,h]), woT[D,D] ([h,o]),
         outT[D,T] ([o,t]).
    TB: t-block size for the attention phase. CH: x-streaming chunk size.
    """
    from contextlib import ExitStack

    import concourse.mybir as mybir
    from concourse import bass_isa
    from concourse.bass import ts

    nc = tc.nc
    fp32 = mybir.dt.float32
    bf16 = mybir.dt.bfloat16
    Exp = mybir.ActivationFunctionType.Exp

    XO = D // P          # x (contraction) tiles
    HO = D // P          # h tiles
    SO = T // P          # s tiles
    NTB = T // TB        # t blocks
    NCH = T // CH        # x-stream chunks over t/s
    VH = min(512, D)     # v-proj h chunk
    NVH = D // VH
    SCALE = float(1.0 / np.sqrt(np.float32(T)))

    xT, wqT, wkT, wvT, woT, outT = (
        aps["xT"], aps["wqT"], aps["wkT"], aps["wvT"], aps["woT"], aps["outT"],
    )

    with ExitStack() as top:
        persist = top.enter_context(tc.tile_pool(name="persist", bufs=1))

        kT = persist.tile([P, HO, T], bf16, name="kT", tag="kT")
        qT = persist.tile([P, HO, T], bf16, name="qT", tag="qT")
        vsb = persist.tile([P, SO, D], bf16, name="vsb", tag="vsb")
        # ---------------- phase 1: q/k/v projections (x streamed once) ------
        # Inputs arrive bf16 from the host; weight loads are interleaved with
        # the first chunk's matmuls so the PE starts as early as possible.
        with ExitStack() as ph1:
            xs_pool = ph1.enter_context(tc.tile_pool(name="xs", bufs=2))
            w_pool = ph1.enter_context(tc.tile_pool(name="w", bufs=1))
            ps1 = ph1.enter_context(tc.tile_pool(name="ps1", bufs=4, space="PSUM"))

            wq = [w_pool.tile([P, D], bf16, name=f"wq{x}", tag=f"wq{x}") for x in range(XO)]
            wk = [w_pool.tile([P, D], bf16, name=f"wk{x}", tag=f"wk{x}") for x in range(XO)]
            wv = [w_pool.tile([P, D], bf16, name=f"wv{x}", tag=f"wv{x}") for x in range(XO)]
            xts0 = xs_pool.tile([P, XO, CH], bf16, name="xs0", tag="xs")
            for x in range(XO):
                nc.sync.dma_start(xts0[:, x, :], xT[ts(x, P), ts(0, CH)])
            for x in range(XO):
                nc.sync.dma_start(wq[x][:], wqT[ts(x, P), :])

            def proj_qk(dst, w, xts, i):
                for h in range(HO):
                    ps = ps1.tile([P, CH], fp32, name="ps_qk", tag="ps")
                    for x in range(XO):
                        nc.tensor.matmul(
                            ps[:], w[x][:, ts(h, P)], xts[:, x, :],
                            start=(x == 0), stop=(x == XO - 1),
                        )
                    nc.scalar.copy(dst[:, h, ts(i, CH)], ps[:])

            def proj_v(xts, i):
                for sl in range(CH // P):
                    s = i * (CH // P) + sl
                    for hc in range(NVH):
                        vps = ps1.tile([P, VH], fp32, name="vps", tag="ps")
                        for x in range(XO):
                            nc.tensor.matmul(
                                vps[:], xts[:, x, ts(sl, P)], wv[x][:, ts(hc, VH)],
                                start=(x == 0), stop=(x == XO - 1),
                            )
                        nc.vector.tensor_copy(vsb[:, s, ts(hc, VH)], vps[:])

            for i in range(NCH):
                if i == 0:
                    xts = xts0
                else:
                    xts = xs_pool.tile([P, XO, CH], bf16, name=f"xs{i}", tag="xs")
                    for x in range(XO):
                        nc.sync.dma_start(xts[:, x, :], xT[ts(x, P), ts(i, CH)])
                proj_qk(qT, wq, xts, i)
                if i == 0:
                    for x in range(XO):
                        nc.sync.dma_start(wk[x][:], wkT[ts(x, P), :])
                proj_qk(kT, wk, xts, i)
                if i == 0:
                    for x in range(XO):
                        nc.sync.dma_start(wv[x][:], wvT[ts(x, P), :])
                proj_v(xts, i)

        # ---------------- phase 2: attention + output projection ------------
        with ExitStack() as ph2:
            wo_pool = ph2.enter_context(tc.tile_pool(name="wo", bufs=1))
            exp_pool = ph2.enter_context(tc.tile_pool(name="expp", bufs=SO + 2))
            ot_pool = ph2.enter_context(tc.tile_pool(name="ot", bufs=HO + 2))
            out_pool = ph2.enter_context(tc.tile_pool(name="outp", bufs=4))
            rc_pool = ph2.enter_context(tc.tile_pool(name="rc", bufs=2))
            acc_pool = ph2.enter_context(tc.tile_pool(name="accp", bufs=2))
            ps_s = ph2.enter_context(tc.tile_pool(name="pss", bufs=2, space="PSUM"))
            ps_o = ph2.enter_context(tc.tile_pool(name="pso", bufs=3, space="PSUM"))
            ps_w = ph2.enter_context(tc.tile_pool(name="psw", bufs=2, space="PSUM"))

            wo = [wo_pool.tile([P, D], bf16, name=f"wo{h}", tag=f"wo{h}") for h in range(HO)]
            for h in range(HO):
                nc.sync.dma_start(wo[h][:], woT[ts(h, P), :])

            for tb in range(NTB):
                # scores + exp, s-tile at a time; denominator accumulates on
                # DVE and reduces across partitions on the (idle) GpSimd
                acc = acc_pool.tile([P, TB], fp32, name="acc", tag="acc")
                exps = []
                for s in range(SO):
                    sps = ps_s.tile([P, TB], fp32, name="sps", tag="sps")
                    for h in range(HO):
                        nc.tensor.matmul(
                            sps[:], kT[:, h, ts(s, P)], qT[:, h, ts(tb, TB)],
                            start=(h == 0), stop=(h == HO - 1),
                        )
                    et = exp_pool.tile([P, TB], bf16, name=f"exp{s}", tag="exp")
                    nc.scalar.activation(et[:], sps[:], Exp, scale=SCALE)
                    exps.append(et)
                    if s == 0:
                        nc.vector.tensor_copy(acc[:], et[:])
                    else:
                        nc.vector.tensor_add(acc[:], acc[:], et[:])

                denom = acc_pool.tile([P, TB], fp32, name="denom", tag="denom")
                nc.gpsimd.partition_all_reduce(
                    denom[:], acc[:], channels=P, reduce_op=bass_isa.ReduceOp.add
                )
                recip = rc_pool.tile([P, TB], fp32, name="recip", tag="recip")
                nc.vector.reciprocal(recip[:], denom[:])

                # oT[h,:] = sum_s v[s,h-slice].T @ expT[s], then normalize
                ots = []
                for h in range(HO):
                    ops = ps_o.tile([P, TB], fp32, name="ops", tag="ops")
                    for s in range(SO):
                        nc.tensor.matmul(
                            ops[:], vsb[:, s, ts(h, P)], exps[s][:],
                            start=(s == 0), stop=(s == SO - 1),
                        )
                    ot = ot_pool.tile([P, TB], bf16, name=f"ot{h}", tag="ot")
                    nc.vector.tensor_mul(ot[:], ops[:], recip[:])
                    ots.append(ot)

                # output projection: outT[o,:] = sum_h woT[h,o-slice].T @ oT[h]
                for o in range(HO):
                    wps = ps_w.tile([P, TB], fp32, name="wps", tag="wps")
                    for h in range(HO):
                        nc.tensor.matmul(
                            wps[:], wo[h][:, ts(o, P)], ots[h][:],
                            start=(h == 0), stop=(h == HO - 1),
                        )
                    osb = out_pool.tile([P, TB], fp32, name="osb", tag="osb")
                    nc.scalar.copy(osb[:], wps[:])
                    nc.sync.dma_start(outT[ts(o, P), ts(tb, TB)], osb[:])


def build_bass(D=1024, T=2048, TB=512, CH=512):
    import concourse.mybir as mybir
    import concourse.tile as tile
    from concourse import bacc

    fp32 = mybir.dt.float32
    bf16 = mybir.dt.bfloat16
    nc = bacc.Bacc("TRN2", debug=False)
    aps = {
        "xT": nc.dram_tensor("xT", [D, T], bf16, kind="ExternalInput")[:],
        "wqT": nc.dram_tensor("wqT", [D, D], bf16, kind="ExternalInput")[:],
        "wkT": nc.dram_tensor("wkT", [D, D], bf16, kind="ExternalInput")[:],
        "wvT": nc.dram_tensor("wvT", [D, D], bf16, kind="ExternalInput")[:],
        "woT": nc.dram_tensor("woT", [D, D], bf16, kind="ExternalInput")[:],
        "outT": nc.dram_tensor("outT", [D, T], fp32, kind="ExternalOutput")[:],
    }
    with tile.TileContext(nc) as tc:
        _build_attention(tc, aps, D=D, T=T, TB=TB, CH=CH)
    nc.compile()
    return nc


def kernel(x, W_q, W_k, W_v, W_o):
    from concourse import bass_utils

    import ml_dtypes

    bf16 = ml_dtypes.bfloat16
    x = np.asarray(x, dtype=np.float32)
    B = x.shape[0]
    wqT = np.ascontiguousarray(np.asarray(W_q, np.float32).T.astype(bf16))
    wkT = np.ascontiguousarray(np.asarray(W_k, np.float32).T.astype(bf16))
    wvT = np.ascontiguousarray(np.asarray(W_v, np.float32).T.astype(bf16))
    woT = np.ascontiguousarray(np.asarray(W_o, np.float32).T.astype(bf16))

    in_maps = [
        {
            "xT": np.ascontiguousarray(x[b].T.astype(bf16)),
            "wqT": wqT,
            "wkT": wkT,
            "wvT": wvT,
            "woT": woT,
        }
        for b in range(B)
    ]

    nc = build_bass()
    res = bass_utils.run_bass_kernel_spmd(nc, in_maps, core_ids=list(range(B)))
    out = np.stack([res.results[b]["outT"].T for b in range(B)])
    return np.ascontiguousarray(out.astype(np.float32))


# revision 11
# speedup vs baseline: 1.0556x; 1.0210x over previous
"""Single-head attention layer on 8 NeuronCores, data-parallel over batch.

Per core (one batch): x [T, D] with T=2048, D=1024.
    q = x@Wq.T, k = x@Wk.T, v = x@Wv.T
    score = q@k.T / sqrt(T); attn = softmax(score); out = (attn@v)@Wo.T

Everything on-chip is kept feature-major (transposed), so no transposes are
ever needed on the device (the host pre-transposes x and the weights, and
re-transposes the output):
    qT[h,t] = wqT.T @ xT          kT[h,s] = wkT.T @ xT
    v[s,h]  = xT.T @ wvT          scoreT[s,t] = kT_slice.T @ qT
    expT    = exp(scoreT/sqrt(T))            (no max subtraction: |score/sqrt(T)|<~5)
    denom   = partition_all_reduce(sum_s expT)   (softmax denominator on DVE+GpSimd,
                                                  result broadcast on all partitions)
    oT[h,t] = v_slice.T @ expT;  oT *= 1/denom
    outT[o,t] = woT_slice.T @ oT

All matmul operands are bf16 (inputs are DMA-cast fp32->bf16 on load), so
every matmul runs at 1 cycle/row with fast weight load; accumulation is fp32
in PSUM and the softmax normalization arithmetic is fp32.
"""

import numpy as np

P = 128


def _build_attention(tc, aps, D, T, TB, CH):
    """Emit the per-core attention kernel into TileContext `tc`.

    aps: dict with DRAM APs xT[D,T], wqT/wkT/wvT[D,D] ([x,h]), woT[D,D] ([h,o]),
         outT[D,T] ([o,t]).
    TB: t-block size for the attention phase. CH: x-streaming chunk size.
    """
    from contextlib import ExitStack

    import concourse.mybir as mybir
    from concourse import bass_isa
    from concourse.bass import ts

    nc = tc.nc
    fp32 = mybir.dt.float32
    bf16 = mybir.dt.bfloat16
    Exp = mybir.ActivationFunctionType.Exp

    XO = D // P          # x (contraction) tiles
    HO = D // P          # h tiles
    SO = T // P          # s tiles
    NTB = T // TB        # t blocks
    NCH = T // CH        # x-stream chunks over t/s
    VH = min(512, D)     # v-proj h chunk
    NVH = D // VH
    SCALE = float(1.0 / np.sqrt(np.float32(T)))

    xT, wqT, wkT, wvT, woT, outT = (
        aps["xT"], aps["wqT"], aps["wkT"], aps["wvT"], aps["woT"], aps["outT"],
    )

    with ExitStack() as top:
        persist = top.enter_context(tc.tile_pool(name="persist", bufs=1))

        kT = persist.tile([P, HO, T], bf16, name="kT", tag="kT")
        qT = persist.tile([P, HO, T], bf16, name="qT", tag="qT")
        vsb = persist.tile([P, SO, D], bf16, name="vsb", tag="vsb")
        # ---------------- phase 1: q/k/v projections (x streamed once) ------
        # Inputs arrive bf16 from the host; weight loads are interleaved with
        # the first chunk's matmuls so the PE starts as early as possible.
        with ExitStack() as ph1:
            xs_pool = ph1.enter_context(tc.tile_pool(name="xs", bufs=2))
            w_pool = ph1.enter_context(tc.tile_pool(name="w", bufs=1))
            ps1 = ph1.enter_context(tc.tile_pool(name="ps1", bufs=4, space="PSUM"))

            wq = [w_pool.tile([P, D], bf16, name=f"wq{x}", tag=f"wq{x}") for x in range(XO)]
            wk = [w_pool.tile([P, D], bf16, name=f"wk{x}", tag=f"wk{x}") for x in range(XO)]
            wv = [w_pool.tile([P, D], bf16, name=f"wv{x}", tag=f"wv{x}") for x in range(XO)]
            xts0 = xs_pool.tile([P, XO, CH], bf16, name="xs0", tag="xs")
            for x in range(XO):
                nc.sync.dma_start(xts0[:, x, :], xT[ts(x, P), ts(0, CH)])
            for x in range(XO):
                nc.sync.dma_start(wq[x][:], wqT[ts(x, P), :])

            def proj_qk(dst, w, xts, i):
                for h in range(HO):
                    ps = ps1.tile([P, CH], fp32, name="ps_qk", tag="ps")
                    for x in range(XO):
                        nc.tensor.matmul(
                            ps[:], w[x][:, ts(h, P)], xts[:, x, :],
                            start=(x == 0), stop=(x == XO - 1),
                        )
                    nc.scalar.copy(dst[:, h, ts(i, CH)], ps[:])

            def proj_v(xts, i):
                for sl in range(CH // P):
                    s = i * (CH // P) + sl
                    for hc in range(NVH):
                        vps = ps1.tile([P, VH], fp32, name="vps", tag="ps")
                        for x in range(XO):
                            nc.tensor.matmul(
                                vps[:], xts[:, x, ts(sl, P)], wv[x][:, ts(hc, VH)],
                                start=(x == 0), stop=(x == XO - 1),
                            )
                        nc.vector.tensor_copy(vsb[:, s, ts(hc, VH)], vps[:])

            for i in range(NCH):
                if i == 0:
                    xts = xts0
                else:
                    xts = xs_pool.tile([P, XO, CH], bf16, name=f"xs{i}", tag="xs")
                    for x in range(XO):
                        nc.sync.dma_start(xts[:, x, :], xT[ts(x, P), ts(i, CH)])
                proj_qk(qT, wq, xts, i)
                if i == 0:
                    for x in range(XO):
                        nc.sync.dma_start(wk[x][:], wkT[ts(x, P), :])
                proj_qk(kT, wk, xts, i)
                if i == 0:
                    for x in range(XO):
                        nc.sync.dma_start(wv[x][:], wvT[ts(x, P), :])
                proj_v(xts, i)

        # ---------------- phase 2: attention + output projection ------------
        with ExitStack() as ph2:
            wo_pool = ph2.enter_context(tc.tile_pool(name="wo", bufs=1))
            exp_pool = ph2.enter_context(tc.tile_pool(name="expp", bufs=SO + 2))
            ot_pool = ph2.enter_context(tc.tile_pool(name="ot", bufs=HO + 2))
            out_pool = ph2.enter_context(tc.tile_pool(name="outp", bufs=4))
            rc_pool = ph2.enter_context(tc.tile_pool(name="rc", bufs=2))
            acc_pool = ph2.enter_context(tc.tile_pool(name="accp", bufs=2))
            ps_s = ph2.enter_context(tc.tile_pool(name="pss", bufs=2, space="PSUM"))
            ps_o = ph2.enter_context(tc.tile_pool(name="pso", bufs=3, space="PSUM"))
            ps_w = ph2.enter_context(tc.tile_pool(name="psw", bufs=2, space="PSUM"))

            wo = [wo_pool.tile([P, D], bf16, name=f"wo{h}", tag=f"wo{h}") for h in range(HO)]
            for h in range(HO):
                nc.sync.dma_start(wo[h][:], woT[ts(h, P), :])

            for tb in range(NTB):
                # scores + exp, s-tile at a time. The softmax denominator
                # accumulates on DVE in two halves; each half all-reduces
                # across partitions on the (otherwise idle) GpSimd as soon as
                # it is complete, so the reduce latency hides under the
                # remaining score matmuls.
                HALF = SO // 2
                accs = [
                    acc_pool.tile([P, TB], fp32, name=f"acc{j}", tag=f"acc{j}")
                    for j in range(2)
                ]
                dens = [
                    acc_pool.tile([P, TB], fp32, name=f"den{j}", tag=f"den{j}")
                    for j in range(2)
                ]
                exps = []
                for s in range(SO):
                    sps = ps_s.tile([P, TB], fp32, name="sps", tag="sps")
                    for h in range(HO):
                        nc.tensor.matmul(
                            sps[:], kT[:, h, ts(s, P)], qT[:, h, ts(tb, TB)],
                            start=(h == 0), stop=(h == HO - 1),
                        )
                    et = exp_pool.tile([P, TB], bf16, name=f"exp{s}", tag="exp")
                    nc.scalar.activation(et[:], sps[:], Exp, scale=SCALE)
                    exps.append(et)
                    j, sj = divmod(s, HALF)
                    if sj == 0:
                        nc.vector.tensor_copy(accs[j][:], et[:])
                    else:
                        nc.vector.tensor_add(accs[j][:], accs[j][:], et[:])
                    if sj == HALF - 1:
                        nc.gpsimd.partition_all_reduce(
                            dens[j][:], accs[j][:], channels=P,
                            reduce_op=bass_isa.ReduceOp.add,
                        )

                recip = rc_pool.tile([P, TB], fp32, name="recip", tag="recip")
                nc.vector.tensor_add(recip[:], dens[0][:], dens[1][:])
                nc.vector.reciprocal(recip[:], recip[:])

                # oT[h,:] = sum_s v[s,h-slice].T @ expT[s], then normalize
                ots = []
                for h in range(HO):
                    ops = ps_o.tile([P, TB], fp32, name="ops", tag="ops")
                    for s in range(SO):
                        nc.tensor.matmul(
                            ops[:], vsb[:, s, ts(h, P)], exps[s][:],
                            start=(s == 0), stop=(s == SO - 1),
                        )
                    ot = ot_pool.tile([P, TB], bf16, name=f"ot{h}", tag="ot")
                    nc.vector.tensor_mul(ot[:], ops[:], recip[:])
                    ots.append(ot)

                # output projection: outT[o,:] = sum_h woT[h,o-slice].T @ oT[h]
                for o in range(HO):
                    wps = ps_w.tile([P, TB], fp32, name="wps", tag="wps")
                    for h in range(HO):
                        nc.tensor.matmul(
                            wps[:], wo[h][:, ts(o, P)], ots[h][:],
                            start=(h == 0), stop=(h == HO - 1),
                        )
                    osb = out_pool.tile([P, TB], fp32, name="osb", tag="osb")
                    nc.scalar.copy(osb[:], wps[:])
                    nc.sync.dma_start(outT[ts(o, P), ts(tb, TB)], osb[:])


def build_bass(D=1024, T=2048, TB=512, CH=512):
    import concourse.mybir as mybir
    import concourse.tile as tile
    from concourse import bacc

    fp32 = mybir.dt.float32
    bf16 = mybir.dt.bfloat16
    nc = bacc.Bacc("TRN2", debug=False)
    aps = {
        "xT": nc.dram_tensor("xT", [D, T], bf16, kind="ExternalInput")[:],
        "wqT": nc.dram_tensor("wqT", [D, D], bf16, kind="ExternalInput")[:],
        "wkT": nc.dram_tensor("wkT", [D, D], bf16, kind="ExternalInput")[:],
        "wvT": nc.dram_tensor("wvT", [D, D], bf16, kind="ExternalInput")[:],
        "woT": nc.dram_tensor("woT", [D, D], bf16, kind="ExternalInput")[:],
        "outT": nc.dram_tensor("outT", [D, T], fp32, kind="ExternalOutput")[:],
    }
    with tile.TileContext(nc) as tc:
        _build_attention(tc, aps, D=D, T=T, TB=TB, CH=CH)
    nc.compile()
    return nc


def kernel(x, W_q, W_k, W_v, W_o):
    from concourse import bass_utils

    import ml_dtypes

    bf16 = ml_dtypes.bfloat16
    x = np.asarray(x, dtype=np.float32)
    B = x.shape[0]
    wqT = np.ascontiguousarray(np.asarray(W_q, np.float32).T.astype(bf16))
    wkT = np.ascontiguousarray(np.asarray(W_k, np.float32).T.astype(bf16))
    wvT = np.ascontiguousarray(np.asarray(W_v, np.float32).T.astype(bf16))
    woT = np.ascontiguousarray(np.asarray(W_o, np.float32).T.astype(bf16))

    in_maps = [
        {
            "xT": np.ascontiguousarray(x[b].T.astype(bf16)),
            "wqT": wqT,
            "wkT": wkT,
            "wvT": wvT,
            "woT": woT,
        }
        for b in range(B)
    ]

    nc = build_bass()
    res = bass_utils.run_bass_kernel_spmd(nc, in_maps, core_ids=list(range(B)))
    out = np.stack([res.results[b]["outT"].T for b in range(B)])
    return np.ascontiguousarray(out.astype(np.float32))
